# revision 50
# baseline (speedup 1.0000x reference)
"""MLA (multi-head latent attention) distributed Bass kernel for TRN2.

Full inputs in / full output out. Sharding: 8 cores = 2 batches x 4 head-groups
(4 heads each). Per-core kernel computes the latent down-projections (duplicated
across the 4 cores of a batch), up-projects Q/K/V for its 4 heads, does causal
attention in a transposed [key, query] layout (scores^T from one matmul, exp on
ScalarE with the 1/sqrt(dk) folded into the activation scale, softmax
denominator via a ones-column appended to the V stationary), and a row-sharded
W_o partial product with W_o as the stationary operand (output in [m, q]
layout). Host sums the 4 partials per batch and transposes.

Key perf facts this schedule is built around (measured via NTFF/HAM):
- The PE clock-gate (HAM) halves the PE clock unless the array sees
  sustained full-geometry activity: every matmul here presents a
  [128, x] stationary (QK uses per-head zero-padded K tiles), dummy
  matmuls pre-warm the gate during the input DMA window, and >1us PE
  gaps are scheduled away (a single idle window re-cools the gate).
- Each dma_start costs ~600ns of queue issue time, so xT moves in a
  few MB-scale slices and W_o chunks 0-2 are staged and stored with
  one DMA per chunk.
- ScalarE's queue is in-order: W_o psum->sbuf casts only go to ScalarE
  where its exp backlog is provably drained (the tail), else Vector.
- The softmax reciprocal runs on the 5x-faster approx custom-DVE op;
  operands are staged to partition-0 SBUF (the custom uop misbehaves
  on PSUM/offset operands).

Per head, attention runs kb-steps with a LAG=2 software pipeline:
QK+exp run LAG steps ahead of the AV sweep; projection/W_o pieces
interleave into the streams as 'extras' keyed by (head, cp, step).
"""

import math
import sys
import types
import numpy as np
import ml_dtypes

import concourse.bass as bass
import concourse.bacc as bacc
import concourse.mybir as mybir
import concourse.tile as tile
from concourse import bass_utils


def _harden_trace_path():
    """The agent image's antenv lacks axon_hooks and has no artifact
    bucket; if the caller enables tracing (e.g. BASS_TRACE=1), the
    bass_utils axon path would crash. Fill both gaps defensively."""
    try:
        import antenv
        try:
            import antenv.axon_hooks  # noqa: F401
        except ImportError:
            hooks = types.ModuleType("antenv.axon_hooks")
            hooks._hook = None
            hooks.set_axon_ntff_profile_hook = (
                lambda h: setattr(hooks, "_hook", h))
            hooks.get_axon_ntff_profile_hook = lambda: hooks._hook
            sys.modules["antenv.axon_hooks"] = hooks
            antenv.axon_hooks = hooks
            try:
                from trn_agent_boot.trn_boot import _ntff_profile_via_ctypes
                hook = _ntff_profile_via_ctypes("/opt/axon/libaxon_pjrt.so")
                if hook is not None:
                    hooks.set_axon_ntff_profile_hook(hook)
            except Exception:
                pass
    except ImportError:
        pass
    orig_upload = bass_utils.upload_artifacts

    def _safe_upload(tmpdir):
        try:
            return orig_upload(tmpdir)
        except Exception:
            return tmpdir

    bass_utils.upload_artifacts = _safe_upload


_harden_trace_path()

BF16 = ml_dtypes.bfloat16

D_MODEL = 1024
N_HEADS = 16
D_K = 64
D_C = 256
B, S = 2, 2048

NH = 4          # heads per core
CH = 512        # query chunk (psum bank)
NCH = S // CH   # 4 query chunks
P = 128
NKB = S // P    # 16 key blocks
INV_SQRT_DK = 1.0 / math.sqrt(D_K)

_cached = None


def build_kernel():
    nc = bacc.Bacc("TRN2", debug=False, num_devices=8)
    dt = mybir.dt
    EXP = mybir.ActivationFunctionType.Exp
    NKD = D_MODEL // P  # 8 d_model blocks

    xT_d = nc.dram_tensor("xT", [D_MODEL, S], dt.bfloat16, kind="ExternalInput")
    aq_d = nc.dram_tensor("aq", [P, NKD, NH * D_K], dt.bfloat16, kind="ExternalInput")
    wdkv_d = nc.dram_tensor("wdkv", [P, 2, NKD, P], dt.bfloat16, kind="ExternalInput")
    wuk_d = nc.dram_tensor("wuk", [P, D_C // P, NH * D_K], dt.bfloat16, kind="ExternalInput")
    wuv_d = nc.dram_tensor("wuv", [P, D_C // P, NH * D_K], dt.bfloat16, kind="ExternalInput")
    wo_d = nc.dram_tensor("wo", [2, P, D_MODEL], dt.bfloat16, kind="ExternalInput")
    tri_d = nc.dram_tensor("tri", [P, P], dt.bfloat16, kind="ExternalInput")
    # output: y^T = (x @ ... @ W_o)^T in [m, q] layout
    yT_d = nc.dram_tensor("yT", [D_MODEL, S], dt.bfloat16, kind="ExternalOutput")

    with tile.TileContext(nc) as tc:
        with (
            tc.tile_pool(name="const", bufs=1) as const,
            tc.tile_pool(name="acts", bufs=1) as acts,
            tc.tile_pool(name="exps", bufs=1) as exps,
            tc.tile_pool(name="work", bufs=4) as work,
            tc.tile_pool(name="ps", bufs=2, space="PSUM") as ps,
            tc.tile_pool(name="psa", bufs=2, space="PSUM") as psa,
            tc.tile_pool(name="psqk", bufs=2, space="PSUM") as psqk,
        ):
            xTv = xT_d.ap().rearrange("(n p) s -> p n s", p=P)
            # loads, first-needed first. Each dma_start costs ~600ns of Sync
            # queue issue time, so xT moves in 4 big column-quarter DMAs
            # (1 MB each) rather than per-d_model-block slices.
            wdkv = const.tile([P, 2, NKD, P], dt.bfloat16, tag="wdkv")
            nc.sync.dma_start(wdkv[:, 0], wdkv_d.ap()[:, 0])
            xT = const.tile([P, NKD, S], dt.bfloat16, tag="xT")
            # first chunk in 4 slices so ckv(0,0)'s k=0 matmul starts after
            # ~0.5 MB instead of a whole 1 MB quarter
            for n0 in range(0, NKD, 2):
                nc.sync.dma_start(xT[:, n0:n0 + 2, 0:CH],
                                  xTv[:, n0:n0 + 2, 0:CH])
            nc.sync.dma_start(wdkv[:, 1], wdkv_d.ap()[:, 1])
            for n0 in range(0, NKD, 4):
                nc.sync.dma_start(xT[:, n0:n0 + 4, CH:2 * CH],
                                  xTv[:, n0:n0 + 4, CH:2 * CH])
            wuk = const.tile([P, D_C // P, NH * D_K], dt.bfloat16, tag="wuk")
            nc.sync.dma_start(wuk[:], wuk_d.ap())
            aq = const.tile([P, NKD, NH * D_K], dt.bfloat16, tag="aq")
            nc.sync.dma_start(aq[:], aq_d.ap())
            wuv = const.tile([P, D_C // P, NH * D_K], dt.bfloat16, tag="wuv")
            nc.sync.dma_start(wuv[:], wuv_d.ap())
            tri = const.tile([P, P], dt.bfloat16, tag="tri")
            nc.sync.dma_start(tri[:], tri_d.ap())
            nc.sync.dma_start(xT[:, :, 2 * CH:3 * CH], xTv[:, :, 2 * CH:3 * CH])
            nc.sync.dma_start(xT[:, :, 3 * CH:S], xTv[:, :, 3 * CH:S])
            wo = []
            for n in range(2):
                t = const.tile([P, D_MODEL], dt.bfloat16, name=f"wo{n}", tag=f"wo{n}")
                nc.sync.dma_start(t[:], wo_d.ap()[n])
                wo.append(t)

            # pre-warm the PE while input DMAs are in flight: the HAM clock
            # gate needs ~3.4us of sustained full-array activity to lift the
            # PE from 1.2 to 2.4 GHz, so burn the DMA dead time on dummy
            # matmuls over an uninitialized scratch tile (result never read)
            scratch = const.tile([P, 5 * P], dt.bfloat16, tag="scratch")
            nc.gpsimd.memset(scratch[:], 1.0)
            psd = psqk.tile([P, 2 * CH], dt.float32, name="pwarm", tag="qk")
            for _ in range(22):
                nc.tensor.matmul(psd[:, 0:CH], scratch[:, 0:P],
                                 scratch[:, P:5 * P], start=True, stop=True)

            # persistent activations
            ckvT = [acts.tile([P, S], dt.bfloat16, name=f"ckvT{i}", tag=f"ckvT{i}")
                    for i in range(2)]
            # fp8 Q^T per head pair m: rows 64j hold head 2m+j. fp8 runs at
            # bf16 speed at this size. K^T is stored zero-padded to 128 rows
            # per head (head 2m+j in rows 64j, zeros elsewhere): the QK
            # stationary is then a full [128,128] tile (moving = both heads'
            # Q rows, zeros cancel the other head), so every attention matmul
            # drives the full PE array and the HAM clock-gate stays warm
            # (32-row DoubleRow stationaries read as idle -> 1.2 GHz).
            qTp = [acts.tile([P, S], dt.float8e4, name=f"qTp{m}", tag=f"qTp{m}")
                   for m in range(2)]
            kz = [[acts.tile([P, S], dt.float8e4, name=f"kz{m}{j}",
                             tag=f"kz{m}{j}") for j in range(2)]
                  for m in range(2)]
            for m in range(2):
                nc.gpsimd.memset(kz[m][0][D_K:P, :], 0.0)
                nc.gpsimd.memset(kz[m][1][0:D_K, :], 0.0)
            v_sb = [None] * NKB
            outT = [acts.tile([P, S], dt.bfloat16, name=f"outT{m}", tag=f"outT{m}")
                    for m in range(2)]

            # ---- single-psum projection pieces (interleavable) ----
            def emit_ckv(ch, half):
                sl = slice(ch * CH, (ch + 1) * CH)
                pp = ps.tile([P, CH], dt.float32, name="pp", tag="ps")
                for k in range(NKD):
                    nc.tensor.matmul(
                        pp[:], wdkv[:, half, k, :],
                        xT[:, k, sl], start=(k == 0), stop=(k == NKD - 1))
                nc.vector.tensor_copy(ckvT[half][:, sl], pp[:])

            def emit_k(ch, m):
                sl = slice(ch * CH, (ch + 1) * CH)
                pp = ps.tile([P, CH], dt.float32, name="pp", tag="ps")
                for half in range(2):
                    nc.tensor.matmul(
                        pp[:], wuk[:, half, m * P:(m + 1) * P],
                        ckvT[half][:, sl], start=(half == 0), stop=(half == 1))
                nc.vector.tensor_copy(kz[m][0][0:D_K, sl], pp[0:D_K, :])
                nc.vector.tensor_copy(kz[m][1][D_K:P, sl], pp[D_K:P, :])

            def emit_q(ch, m):
                sl = slice(ch * CH, (ch + 1) * CH)
                pp = ps.tile([P, CH], dt.float32, name="pp", tag="ps")
                for k in range(NKD):
                    nc.tensor.matmul(
                        pp[:], aq[:, k, m * P:(m + 1) * P],
                        xT[:, k, sl], start=(k == 0), stop=(k == NKD - 1))
                nc.vector.tensor_copy(qTp[m][:, sl], pp[:])

            def emit_v(kb):
                # V in [key, dim]: per head 64 dims + 64-wide ones block
                # (the ones columns replicate the softmax denominator to
                # psum rows 64:128 for free)
                vt = acts.tile([P, NH, 2 * D_K], dt.bfloat16,
                               name=f"v{kb}", tag=f"v{kb}")
                psv = ps.tile([P, NH * D_K], dt.float32, tag="ps")
                for half in range(2):
                    nc.tensor.matmul(
                        psv[:], ckvT[half][:, kb * P:(kb + 1) * P],
                        wuv[:, half, :], start=(half == 0), stop=(half == 1))
                nc.vector.tensor_copy(
                    vt[:, :, 0:D_K],
                    psv[:].rearrange("p (h d) -> p h d", h=NH))
                nc.gpsimd.memset(vt[:, :, D_K:2 * D_K], 1.0)
                v_sb[kb] = vt

            COPY = mybir.ActivationFunctionType.Copy
            yTv = yT_d.ap().rearrange("(n p) s -> p n s", p=P)
            # chunk-batched output staging: one DMA per query chunk for
            # chunks 0-2 (vs 8 small ones each); chunk 3 stays per-block so
            # the tail doesn't wait on a 1 MB store
            ysball = [acts.tile([P, D_MODEL // P, CH], dt.bfloat16,
                                name=f"ysb{i}", tag=f"ysb{i}") for i in range(2)]

            def emit_wo_mb(ch, mb, on_scalar=False):
                # yT[m, q] = sum_d wo[d, m] outT[d, q]: one (m, q-chunk) block
                sl = slice(ch * CH, (ch + 1) * CH)
                if ch == 3:
                    ysb = work.tile([P, CH], dt.bfloat16,
                                    name="ysb", tag="ysb")[:]
                else:
                    ysb = ysball[ch % 2][:, mb, :]
                pp = ps.tile([P, CH], dt.float32, name="pp", tag="ps")
                for db in range(2):
                    nc.tensor.matmul(
                        pp[:], wo[db][:, mb * P:(mb + 1) * P],
                        outT[db][:, sl], start=(db == 0), stop=(db == 1))
                if on_scalar:  # ScalarE takes casts when exp leaves it idle
                    nc.scalar.activation(ysb, pp[:], COPY)
                else:
                    nc.vector.tensor_copy(ysb, pp[:])
                if ch == 3:
                    nc.sync.dma_start(yT_d.ap()[mb * P:(mb + 1) * P, sl], ysb)

            def emit_wo_flush(ch):
                sl = slice(ch * CH, (ch + 1) * CH)
                nc.sync.dma_start(yTv[:, :, sl], ysball[ch % 2][:])

            # minimal pre-attention pass: just what head 0 cp0 needs
            # (pair-0 q/k for queries/keys 0:1024); the rest interleaves
            # into the attention streams below. Piece order tracks DMA
            # arrival so the PE never gaps (a >.5us gap can re-cool the
            # HAM clock gate).
            emit_ckv(0, 0)
            emit_ckv(0, 1)
            emit_ckv(1, 0)
            emit_ckv(1, 1)
            emit_k(0, 0)
            emit_q(0, 0)
            emit_k(1, 0)
            emit_q(1, 0)

            # per-(head, cp) extras: {h: {cp: {kb: [fns]}}}
            extras_map = {h: {0: {}, 1: {}} for h in range(NH)}

            def put(h, cp, kb, fn):
                extras_map[h][cp].setdefault(kb, []).append(fn)

            # h0 cp0: V blocks 0-7 (AV needs v[kb] at step kb+LAG) and
            # pair-0 queries 1024:2048 (h0 cp1 moving operand)
            for kb in range(8):
                put(0, 0, kb, lambda kb=kb: emit_v(kb))
            put(0, 0, 4, lambda: emit_q(2, 0))
            put(0, 0, 6, lambda: emit_q(3, 0))
            # h0 cp1: latent chunks 2-3 + pair-0 keys 1024:2048 (needed at
            # kb8/kb12) + V blocks 8-15
            pieces = [lambda: emit_ckv(2, 0), lambda: emit_ckv(2, 1),
                      lambda: emit_k(2, 0), lambda: emit_ckv(3, 0),
                      lambda: emit_ckv(3, 1), lambda: emit_k(3, 0)]
            for i, pc in enumerate(pieces):
                put(0, 1, i, pc)
            for kb in range(8, NKB):
                put(0, 1, kb, lambda kb=kb: emit_v(kb))
            # h1: pair-1 q/k (heads 2,3 start at h2)
            put(1, 0, 0, lambda: emit_k(0, 1))
            put(1, 0, 1, lambda: emit_q(0, 1))
            put(1, 0, 2, lambda: emit_k(1, 1))
            put(1, 0, 3, lambda: emit_q(1, 1))
            put(1, 1, 0, lambda: emit_k(2, 1))
            put(1, 1, 1, lambda: emit_q(2, 1))
            put(1, 1, 2, lambda: emit_k(3, 1))
            put(1, 1, 3, lambda: emit_q(3, 1))

            # ---- attention: per head, chunk-pair major, QK/exp ahead of a
            # lagged AV sweep; denominator rows 64:128 of psav ----
            LAG = 2
            for h in range(NH):
                ht, off = divmod(h, 2)
                q_h = qTp[h // 2]
                k_h = kz[h // 2][h % 2]
                es_tiles = [None] * NKB
                psav = [None] * NCH

                def emit_qk(kb, cp):
                    q0 = P * kb       # first valid query for this key block
                    pq0 = 1024 * cp   # pair covers q in [pq0, pq0+1024)
                    if es_tiles[kb] is None:
                        es_tiles[kb] = exps.tile(
                            [P, S - q0], dt.bfloat16,
                            name=f"es{kb}", tag=f"es{kb}")
                    es = es_tiles[kb]
                    lo = max(q0, pq0)
                    pqk = psqk.tile([P, 2 * CH], dt.float32,
                                    name="pqk", tag="qk")
                    for ch in (2 * cp, 2 * cp + 1):
                        clo = max(q0, ch * CH)
                        if clo >= (ch + 1) * CH:
                            continue
                        nc.tensor.matmul(
                            pqk[:, clo - pq0:(ch + 1) * CH - pq0],
                            k_h[:, q0:q0 + P],
                            q_h[:, clo:(ch + 1) * CH],
                            start=True, stop=True)
                    nc.scalar.activation(
                        es[:, lo - q0:pq0 + 2 * CH - q0],
                        pqk[:, lo - pq0:2 * CH],
                        EXP, scale=INV_SQRT_DK)
                    if cp == kb // 8:
                        # mask the diagonal [128, 128] triangle (valid f >= p)
                        nc.vector.tensor_mul(es[:, 0:P], es[:, 0:P], tri[:])

                def emit_av(kb, cp):
                    q0 = P * kb
                    for c in (2 * cp, 2 * cp + 1):
                        if kb // 4 > c:
                            continue
                        n0 = max(q0 - CH * c, 0)
                        nc.tensor.matmul(
                            psav[c][:, n0:CH], v_sb[kb][:, h, :],
                            es_tiles[kb][:, CH * c + n0 - q0:
                                         CH * (c + 1) - q0],
                            start=(kb == 0), stop=(kb == 4 * c + 3))
                        if kb == 4 * c + 3:  # chunk done -> normalize
                            # approx reciprocal (~5x faster than exact); the
                            # custom-DVE op wants SBUF operands at partition
                            # 0, so stage the denominators through rb first
                            # (on Scalar for the last head -- its normalize
                            # chains gate the W_o tail and Vector is busy)
                            rb = work.tile([D_K, CH], dt.float32, tag="rb")
                            if h == NH - 1 and c == 3:
                                # Scalar's exp queue is drained by now, and
                                # Vector is busy with W_o casts: staging on
                                # Scalar unblocks the final W_o chunk sooner
                                nc.scalar.activation(
                                    rb[:], psav[c][D_K:2 * D_K, :], COPY)
                            else:
                                nc.vector.tensor_copy(
                                    rb[:], psav[c][D_K:2 * D_K, :])
                            nc.vector.reciprocal_approx_fast(rb[:], rb[:])
                            nc.vector.tensor_mul(
                                outT[ht][off * D_K:(off + 1) * D_K,
                                         c * CH:(c + 1) * CH],
                                psav[c][0:D_K, :], rb[:])

                for cp in range(2):
                    for c in (2 * cp, 2 * cp + 1):
                        psav[c] = psa.tile([P, CH], dt.float32,
                                           name="psav", tag="psa")
                    kmax = 8 * cp + 8
                    extras = extras_map[h][cp]
                    if h == NH - 1 and cp == 1:
                        # W_o rides along head 3 cp1: chunks 0,1 are fully
                        # normalized after h3 cp0; chunk 2 after step 13.
                        # A few ready pieces are held back past the loop so
                        # the PE stays busy while psav[3] normalizes (a >1us
                        # gap would re-cool the HAM clock gate for the tail).
                        jobs = [(c, mb) for c in (0, 1)
                                for mb in range(D_MODEL // P)][:14]
                        for kb, job in zip(range(1, 15), jobs):
                            # all on Vector: a Scalar cast here would queue
                            # in front of this head's remaining exps
                            extras.setdefault(kb, []).append(
                                lambda job=job: emit_wo_mb(job[0], job[1]))
                        extras.setdefault(9, []).append(
                            lambda: emit_wo_flush(0))
                        for i, mb in enumerate(range(3)):
                            extras.setdefault(15 + i, []).append(
                                lambda mb=mb: emit_wo_mb(2, mb))
                    for kb in range(kmax + LAG):
                        for fn in extras.get(kb, ()):
                            fn()
                        if kb < kmax:
                            emit_qk(kb, cp)
                        if kb >= LAG:
                            emit_av(kb - LAG, cp)
            # dep-free pieces first: they overlap psav[3]'s normalize chain.
            # Alternate the psum->sbuf casts between Scalar (idle here) and
            # Vector so the cast is never the tail's rate limiter.
            emit_wo_mb(1, 6)
            emit_wo_mb(1, 7)
            emit_wo_flush(1)
            for mb in range(3, D_MODEL // P):
                emit_wo_mb(2, mb)
            emit_wo_flush(2)
            for mb in range(D_MODEL // P):
                emit_wo_mb(3, mb, on_scalar=(mb % 2 == 0))

    nc.compile()
    return nc


def _fold(w, p=P):
    # [K, M] -> [p, K/p, M] partition-major layout for contiguous DMA
    k, m = w.shape
    return np.ascontiguousarray(w.reshape(k // p, p, m).transpose(1, 0, 2))


def _prep_inputs(x, W_dq, W_uq, W_dkv, W_uk, W_uv, W_o):
    tri = np.triu(np.ones((P, P), dtype=np.float32)).astype(BF16)  # f >= p
    in_maps = []
    for c in range(8):
        b, hg = divmod(c, 4)
        cs = slice(hg * NH * D_K, (hg + 1) * NH * D_K)
        aq = np.asarray(W_dq, np.float32) @ np.asarray(W_uq, np.float32)[:, cs]
        wuk = np.asarray(W_uk, np.float32)[:, cs]
        in_maps.append({
            "xT": np.ascontiguousarray(np.asarray(x)[b].T).astype(BF16),
            "aq": _fold(aq.astype(BF16)),
            # [P, 2, NKD, P]: c-dim half major, for half-granular DMA
            "wdkv": np.ascontiguousarray(
                _fold(np.asarray(W_dkv).astype(BF16))
                .reshape(P, D_MODEL // P, 2, P).transpose(0, 2, 1, 3)),
            "wuk": _fold(wuk.astype(BF16)),
            "wuv": _fold(np.asarray(W_uv)[:, cs].astype(BF16)),
            "wo": np.asarray(W_o)[cs, :].astype(BF16).reshape(2, P, D_MODEL),
            "tri": tri,
        })
    return in_maps


def run(inputs, trace=False, **kw):
    global _cached
    if _cached is None:
        _cached = build_kernel()
    in_maps = _prep_inputs(**inputs)
    res = bass_utils.run_bass_kernel_spmd(
        _cached, in_maps, core_ids=list(range(8)), trace=trace, **kw)
    ys = [res.results[c]["yT"].astype(np.float32) for c in range(8)]
    out = np.stack([
        (ys[0] + ys[1] + ys[2] + ys[3]).T,
        (ys[4] + ys[5] + ys[6] + ys[7]).T,
    ]).astype(np.float32)
    return out, res


def kernel(**inputs):
    out, _ = run(inputs)
    return out



# revision 51
# speedup vs baseline: 1.0049x; 1.0049x over previous
"""MLA (multi-head latent attention) distributed Bass kernel for TRN2.

Full inputs in / full output out. Sharding: 8 cores = 2 batches x 4 head-groups
(4 heads each). Per-core kernel computes the latent down-projections (duplicated
across the 4 cores of a batch), up-projects Q/K/V for its 4 heads, does causal
attention in a transposed [key, query] layout (scores^T from one matmul, exp on
ScalarE with the 1/sqrt(dk) folded into the activation scale, softmax
denominator via a ones-column appended to the V stationary), and a row-sharded
W_o partial product with W_o as the stationary operand (output in [m, q]
layout). Host sums the 4 partials per batch and transposes.

Key perf facts this schedule is built around (measured via NTFF/HAM):
- The PE clock-gate (HAM) halves the PE clock unless the array sees
  sustained full-geometry activity: every matmul here presents a
  [128, x] stationary (QK uses per-head zero-padded K tiles), dummy
  matmuls pre-warm the gate during the input DMA window, and >1us PE
  gaps are scheduled away (a single idle window re-cools the gate).
- Each dma_start costs ~600ns of queue issue time, so xT moves in a
  few MB-scale slices and W_o chunks 0-2 are staged and stored with
  one DMA per chunk.
- ScalarE's queue is in-order: W_o psum->sbuf casts only go to ScalarE
  where its exp backlog is provably drained (the tail), else Vector.
- The softmax reciprocal runs on the 5x-faster approx custom-DVE op;
  operands are staged to partition-0 SBUF (the custom uop misbehaves
  on PSUM/offset operands).

Per head, attention runs kb-steps with a LAG=2 software pipeline:
QK+exp run LAG steps ahead of the AV sweep; projection/W_o pieces
interleave into the streams as 'extras' keyed by (head, cp, step).
"""

import math
import sys
import types
import numpy as np
import ml_dtypes

import concourse.bass as bass
import concourse.bacc as bacc
import concourse.mybir as mybir
import concourse.tile as tile
from concourse import bass_utils


def _harden_trace_path():
    """The agent image's antenv lacks axon_hooks and has no artifact
    bucket; if the caller enables tracing (e.g. BASS_TRACE=1), the
    bass_utils axon path would crash. Fill both gaps defensively."""
    try:
        import antenv
        try:
            import antenv.axon_hooks  # noqa: F401
        except ImportError:
            hooks = types.ModuleType("antenv.axon_hooks")
            hooks._hook = None
            hooks.set_axon_ntff_profile_hook = (
                lambda h: setattr(hooks, "_hook", h))
            hooks.get_axon_ntff_profile_hook = lambda: hooks._hook
            sys.modules["antenv.axon_hooks"] = hooks
            antenv.axon_hooks = hooks
            try:
                from trn_agent_boot.trn_boot import _ntff_profile_via_ctypes
                hook = _ntff_profile_via_ctypes("/opt/axon/libaxon_pjrt.so")
                if hook is not None:
                    hooks.set_axon_ntff_profile_hook(hook)
            except Exception:
                pass
    except ImportError:
        pass
    orig_upload = bass_utils.upload_artifacts

    def _safe_upload(tmpdir):
        try:
            return orig_upload(tmpdir)
        except Exception:
            return tmpdir

    bass_utils.upload_artifacts = _safe_upload


_harden_trace_path()

BF16 = ml_dtypes.bfloat16

D_MODEL = 1024
N_HEADS = 16
D_K = 64
D_C = 256
B, S = 2, 2048

NH = 4          # heads per core
CH = 512        # query chunk (psum bank)
NCH = S // CH   # 4 query chunks
P = 128
NKB = S // P    # 16 key blocks
INV_SQRT_DK = 1.0 / math.sqrt(D_K)

_cached = None


def build_kernel():
    nc = bacc.Bacc("TRN2", debug=False, num_devices=8)
    dt = mybir.dt
    EXP = mybir.ActivationFunctionType.Exp
    NKD = D_MODEL // P  # 8 d_model blocks

    xT_d = nc.dram_tensor("xT", [D_MODEL, S], dt.bfloat16, kind="ExternalInput")
    aq_d = nc.dram_tensor("aq", [P, NKD, NH * D_K], dt.bfloat16, kind="ExternalInput")
    wdkv_d = nc.dram_tensor("wdkv", [P, 2, NKD, P], dt.bfloat16, kind="ExternalInput")
    wuk_d = nc.dram_tensor("wuk", [P, D_C // P, NH * D_K], dt.bfloat16, kind="ExternalInput")
    wuv_d = nc.dram_tensor("wuv", [P, D_C // P, NH * D_K], dt.bfloat16, kind="ExternalInput")
    wo_d = nc.dram_tensor("wo", [2, P, D_MODEL], dt.bfloat16, kind="ExternalInput")
    tri_d = nc.dram_tensor("tri", [P, P], dt.bfloat16, kind="ExternalInput")
    # output: y^T = (x @ ... @ W_o)^T in [m, q] layout
    yT_d = nc.dram_tensor("yT", [D_MODEL, S], dt.bfloat16, kind="ExternalOutput")

    with tile.TileContext(nc) as tc:
        with (
            tc.tile_pool(name="const", bufs=1) as const,
            tc.tile_pool(name="acts", bufs=1) as acts,
            tc.tile_pool(name="exps", bufs=1) as exps,
            tc.tile_pool(name="work", bufs=4) as work,
            tc.tile_pool(name="ps", bufs=2, space="PSUM") as ps,
            tc.tile_pool(name="psa", bufs=2, space="PSUM") as psa,
            tc.tile_pool(name="psqk", bufs=2, space="PSUM") as psqk,
        ):
            xTv = xT_d.ap().rearrange("(n p) s -> p n s", p=P)
            # loads, first-needed first. Each dma_start costs ~600ns of Sync
            # queue issue time, so xT moves in 4 big column-quarter DMAs
            # (1 MB each) rather than per-d_model-block slices.
            wdkv = const.tile([P, 2, NKD, P], dt.bfloat16, tag="wdkv")
            nc.sync.dma_start(wdkv[:, 0], wdkv_d.ap()[:, 0])
            xT = const.tile([P, NKD, S], dt.bfloat16, tag="xT")
            # first chunk in 4 slices so ckv(0,0)'s k=0 matmul starts after
            # ~0.5 MB instead of a whole 1 MB quarter
            for n0 in range(0, NKD, 2):
                nc.sync.dma_start(xT[:, n0:n0 + 2, 0:CH],
                                  xTv[:, n0:n0 + 2, 0:CH])
            nc.sync.dma_start(wdkv[:, 1], wdkv_d.ap()[:, 1])
            for n0 in range(0, NKD, 4):
                nc.sync.dma_start(xT[:, n0:n0 + 4, CH:2 * CH],
                                  xTv[:, n0:n0 + 4, CH:2 * CH])
            wuk = const.tile([P, D_C // P, NH * D_K], dt.bfloat16, tag="wuk")
            nc.sync.dma_start(wuk[:], wuk_d.ap())
            aq = const.tile([P, NKD, NH * D_K], dt.bfloat16, tag="aq")
            nc.sync.dma_start(aq[:], aq_d.ap())
            wuv = const.tile([P, D_C // P, NH * D_K], dt.bfloat16, tag="wuv")
            nc.sync.dma_start(wuv[:], wuv_d.ap())
            tri = const.tile([P, P], dt.bfloat16, tag="tri")
            nc.sync.dma_start(tri[:], tri_d.ap())
            nc.sync.dma_start(xT[:, :, 2 * CH:3 * CH], xTv[:, :, 2 * CH:3 * CH])
            nc.sync.dma_start(xT[:, :, 3 * CH:S], xTv[:, :, 3 * CH:S])
            wo = []
            for n in range(2):
                t = const.tile([P, D_MODEL], dt.bfloat16, name=f"wo{n}", tag=f"wo{n}")
                nc.sync.dma_start(t[:], wo_d.ap()[n])
                wo.append(t)

            # pre-warm the PE while input DMAs are in flight: the HAM clock
            # gate needs ~3.4us of sustained full-array activity to lift the
            # PE from 1.2 to 2.4 GHz, so burn the DMA dead time on dummy
            # matmuls over an uninitialized scratch tile (result never read)
            scratch = const.tile([P, 5 * P], dt.bfloat16, tag="scratch")
            nc.gpsimd.memset(scratch[:], 1.0)
            psd = psqk.tile([P, 2 * CH], dt.float32, name="pwarm", tag="qk")
            for _ in range(22):
                nc.tensor.matmul(psd[:, 0:CH], scratch[:, 0:P],
                                 scratch[:, P:5 * P], start=True, stop=True)

            # persistent activations
            ckvT = [acts.tile([P, S], dt.bfloat16, name=f"ckvT{i}", tag=f"ckvT{i}")
                    for i in range(2)]
            # fp8 Q^T per head pair m: rows 64j hold head 2m+j. fp8 runs at
            # bf16 speed at this size. K^T is stored zero-padded to 128 rows
            # per head (head 2m+j in rows 64j, zeros elsewhere): the QK
            # stationary is then a full [128,128] tile (moving = both heads'
            # Q rows, zeros cancel the other head), so every attention matmul
            # drives the full PE array and the HAM clock-gate stays warm
            # (32-row DoubleRow stationaries read as idle -> 1.2 GHz).
            qTp = [acts.tile([P, S], dt.float8e4, name=f"qTp{m}", tag=f"qTp{m}")
                   for m in range(2)]
            kz = [[acts.tile([P, S], dt.float8e4, name=f"kz{m}{j}",
                             tag=f"kz{m}{j}") for j in range(2)]
                  for m in range(2)]
            for m in range(2):
                nc.gpsimd.memset(kz[m][0][D_K:P, :], 0.0)
                nc.gpsimd.memset(kz[m][1][0:D_K, :], 0.0)
            v_sb = [None] * NKB
            outT = [acts.tile([P, S], dt.bfloat16, name=f"outT{m}", tag=f"outT{m}")
                    for m in range(2)]

            # ---- single-psum projection pieces (interleavable) ----
            def emit_ckv(ch, half):
                sl = slice(ch * CH, (ch + 1) * CH)
                pp = ps.tile([P, CH], dt.float32, name="pp", tag="ps")
                for k in range(NKD):
                    nc.tensor.matmul(
                        pp[:], wdkv[:, half, k, :],
                        xT[:, k, sl], start=(k == 0), stop=(k == NKD - 1))
                nc.vector.tensor_copy(ckvT[half][:, sl], pp[:])

            def emit_k(ch, m):
                sl = slice(ch * CH, (ch + 1) * CH)
                pp = ps.tile([P, CH], dt.float32, name="pp", tag="ps")
                for half in range(2):
                    nc.tensor.matmul(
                        pp[:], wuk[:, half, m * P:(m + 1) * P],
                        ckvT[half][:, sl], start=(half == 0), stop=(half == 1))
                nc.vector.tensor_copy(kz[m][0][0:D_K, sl], pp[0:D_K, :])
                nc.vector.tensor_copy(kz[m][1][D_K:P, sl], pp[D_K:P, :])

            def emit_q(ch, m):
                sl = slice(ch * CH, (ch + 1) * CH)
                pp = ps.tile([P, CH], dt.float32, name="pp", tag="ps")
                for k in range(NKD):
                    nc.tensor.matmul(
                        pp[:], aq[:, k, m * P:(m + 1) * P],
                        xT[:, k, sl], start=(k == 0), stop=(k == NKD - 1))
                nc.vector.tensor_copy(qTp[m][:, sl], pp[:])

            def emit_v(kb):
                # V in [key, dim]: per head 64 dims + 64-wide ones block
                # (the ones columns replicate the softmax denominator to
                # psum rows 64:128 for free)
                vt = acts.tile([P, NH, 2 * D_K], dt.bfloat16,
                               name=f"v{kb}", tag=f"v{kb}")
                psv = ps.tile([P, NH * D_K], dt.float32, tag="ps")
                for half in range(2):
                    nc.tensor.matmul(
                        psv[:], ckvT[half][:, kb * P:(kb + 1) * P],
                        wuv[:, half, :], start=(half == 0), stop=(half == 1))
                nc.vector.tensor_copy(
                    vt[:, :, 0:D_K],
                    psv[:].rearrange("p (h d) -> p h d", h=NH))
                nc.gpsimd.memset(vt[:, :, D_K:2 * D_K], 1.0)
                v_sb[kb] = vt

            COPY = mybir.ActivationFunctionType.Copy
            yTv = yT_d.ap().rearrange("(n p) s -> p n s", p=P)
            # chunk-batched output staging: one DMA per query chunk for
            # chunks 0-2 (vs 8 small ones each); chunk 3 stays per-block so
            # the tail doesn't wait on a 1 MB store
            ysball = [acts.tile([P, D_MODEL // P, CH], dt.bfloat16,
                                name=f"ysb{i}", tag=f"ysb{i}") for i in range(2)]

            def emit_wo_mb(ch, mb, on_scalar=False):
                # yT[m, q] = sum_d wo[d, m] outT[d, q]: one (m, q-chunk) block
                sl = slice(ch * CH, (ch + 1) * CH)
                if ch == 3:
                    ysb = work.tile([P, CH], dt.bfloat16,
                                    name="ysb", tag="ysb")[:]
                else:
                    ysb = ysball[ch % 2][:, mb, :]
                pp = ps.tile([P, CH], dt.float32, name="pp", tag="ps")
                for db in range(2):
                    nc.tensor.matmul(
                        pp[:], wo[db][:, mb * P:(mb + 1) * P],
                        outT[db][:, sl], start=(db == 0), stop=(db == 1))
                if on_scalar:  # ScalarE takes casts when exp leaves it idle
                    nc.scalar.activation(ysb, pp[:], COPY)
                else:
                    nc.vector.tensor_copy(ysb, pp[:])
                if ch == 3:
                    nc.sync.dma_start(yT_d.ap()[mb * P:(mb + 1) * P, sl], ysb)

            def emit_wo_flush(ch):
                sl = slice(ch * CH, (ch + 1) * CH)
                nc.sync.dma_start(yTv[:, :, sl], ysball[ch % 2][:])

            # minimal pre-attention pass: just what head 0 cp0 needs
            # (pair-0 q/k for queries/keys 0:1024); the rest interleaves
            # into the attention streams below. Piece order tracks DMA
            # arrival so the PE never gaps (a >.5us gap can re-cool the
            # HAM clock gate).
            emit_ckv(0, 0)
            emit_ckv(0, 1)
            emit_ckv(1, 0)
            emit_ckv(1, 1)
            emit_k(0, 0)
            emit_q(0, 0)
            emit_k(1, 0)
            emit_q(1, 0)

            # per-(head, cp) extras: {h: {cp: {kb: [fns]}}}
            extras_map = {h: {0: {}, 1: {}} for h in range(NH)}

            def put(h, cp, kb, fn):
                extras_map[h][cp].setdefault(kb, []).append(fn)

            # h0 cp0: V blocks 0-7 (AV needs v[kb] at step kb+LAG) and
            # pair-0 queries 1024:2048 (h0 cp1 moving operand)
            for kb in range(8):
                put(0, 0, kb, lambda kb=kb: emit_v(kb))
            put(0, 0, 4, lambda: emit_q(2, 0))
            put(0, 0, 6, lambda: emit_q(3, 0))
            # h0 cp1: latent chunks 2-3 + pair-0 keys 1024:2048 (needed at
            # kb8/kb12) + V blocks 8-15
            pieces = [lambda: emit_ckv(2, 0), lambda: emit_ckv(2, 1),
                      lambda: emit_k(2, 0), lambda: emit_ckv(3, 0),
                      lambda: emit_ckv(3, 1), lambda: emit_k(3, 0)]
            for i, pc in enumerate(pieces):
                put(0, 1, i, pc)
            for kb in range(8, NKB):
                put(0, 1, kb, lambda kb=kb: emit_v(kb))
            # h1: pair-1 q/k (heads 2,3 start at h2)
            put(1, 0, 0, lambda: emit_k(0, 1))
            put(1, 0, 1, lambda: emit_q(0, 1))
            put(1, 0, 2, lambda: emit_k(1, 1))
            put(1, 0, 3, lambda: emit_q(1, 1))
            put(1, 1, 0, lambda: emit_k(2, 1))
            put(1, 1, 1, lambda: emit_q(2, 1))
            put(1, 1, 2, lambda: emit_k(3, 1))
            put(1, 1, 3, lambda: emit_q(3, 1))

            # ---- attention: per head, chunk-pair major, QK/exp ahead of a
            # lagged AV sweep; denominator rows 64:128 of psav ----
            LAG = 2
            for h in range(NH):
                ht, off = divmod(h, 2)
                q_h = qTp[h // 2]
                k_h = kz[h // 2][h % 2]
                es_tiles = [None] * NKB
                psav = [None] * NCH

                def emit_qk(kb, cp):
                    q0 = P * kb       # first valid query for this key block
                    pq0 = 1024 * cp   # pair covers q in [pq0, pq0+1024)
                    if es_tiles[kb] is None:
                        es_tiles[kb] = exps.tile(
                            [P, S - q0], dt.bfloat16,
                            name=f"es{kb}", tag=f"es{kb}")
                    es = es_tiles[kb]
                    lo = max(q0, pq0)
                    pqk = psqk.tile([P, 2 * CH], dt.float32,
                                    name="pqk", tag="qk")
                    for ch in (2 * cp, 2 * cp + 1):
                        clo = max(q0, ch * CH)
                        if clo >= (ch + 1) * CH:
                            continue
                        nc.tensor.matmul(
                            pqk[:, clo - pq0:(ch + 1) * CH - pq0],
                            k_h[:, q0:q0 + P],
                            q_h[:, clo:(ch + 1) * CH],
                            start=True, stop=True)
                    nc.scalar.activation(
                        es[:, lo - q0:pq0 + 2 * CH - q0],
                        pqk[:, lo - pq0:2 * CH],
                        EXP, scale=INV_SQRT_DK)
                    if cp == kb // 8:
                        # mask the diagonal [128, 128] triangle (valid f >= p)
                        nc.vector.tensor_mul(es[:, 0:P], es[:, 0:P], tri[:])

                def emit_av(kb, cp):
                    q0 = P * kb
                    for c in (2 * cp, 2 * cp + 1):
                        if kb // 4 > c:
                            continue
                        n0 = max(q0 - CH * c, 0)
                        nc.tensor.matmul(
                            psav[c][:, n0:CH], v_sb[kb][:, h, :],
                            es_tiles[kb][:, CH * c + n0 - q0:
                                         CH * (c + 1) - q0],
                            start=(kb == 0), stop=(kb == 4 * c + 3))
                        if kb == 4 * c + 3:  # chunk done -> normalize
                            # approx reciprocal (~5x faster than exact); the
                            # custom-DVE op wants SBUF operands at partition
                            # 0, so stage the denominators through rb first
                            # (on Scalar for the last head -- its normalize
                            # chains gate the W_o tail and Vector is busy)
                            rb = work.tile([D_K, CH], dt.float32, tag="rb")
                            if h == NH - 1 and c == 3:
                                # Scalar's exp queue is drained by now, and
                                # Vector is busy with W_o casts: staging on
                                # Scalar unblocks the final W_o chunk sooner
                                nc.scalar.activation(
                                    rb[:], psav[c][D_K:2 * D_K, :], COPY)
                            else:
                                nc.vector.tensor_copy(
                                    rb[:], psav[c][D_K:2 * D_K, :])
                            nc.vector.reciprocal_approx_fast(rb[:], rb[:])
                            nc.vector.tensor_mul(
                                outT[ht][off * D_K:(off + 1) * D_K,
                                         c * CH:(c + 1) * CH],
                                psav[c][0:D_K, :], rb[:])

                for cp in range(2):
                    for c in (2 * cp, 2 * cp + 1):
                        psav[c] = psa.tile([P, CH], dt.float32,
                                           name="psav", tag="psa")
                    kmax = 8 * cp + 8
                    extras = extras_map[h][cp]
                    if h == NH - 1 and cp == 1:
                        # W_o rides along head 3 cp1: chunks 0,1 are fully
                        # normalized after h3 cp0; chunk 2 after step 13.
                        # A few ready pieces are held back past the loop so
                        # the PE stays busy while psav[3] normalizes (a >1us
                        # gap would re-cool the HAM clock gate for the tail).
                        jobs = [(c, mb) for c in (0, 1)
                                for mb in range(D_MODEL // P)][:14]
                        for kb, job in zip(range(1, 15), jobs):
                            # all on Vector: a Scalar cast here would queue
                            # in front of this head's remaining exps
                            extras.setdefault(kb, []).append(
                                lambda job=job: emit_wo_mb(job[0], job[1]))
                        extras.setdefault(9, []).append(
                            lambda: emit_wo_flush(0))
                        for i, mb in enumerate(range(3)):
                            extras.setdefault(15 + i, []).append(
                                lambda mb=mb: emit_wo_mb(2, mb))
                    for kb in range(kmax + LAG):
                        for fn in extras.get(kb, ()):
                            fn()
                        if kb < kmax:
                            emit_qk(kb, cp)
                        if kb >= LAG:
                            emit_av(kb - LAG, cp)
            # dep-free pieces first: they overlap psav[3]'s normalize chain.
            # Alternate the psum->sbuf casts between Scalar (idle here) and
            # Vector so the cast is never the tail's rate limiter.
            # Scalar's queue here is [rb-c3 staging copy, ...]: giving it the
            # wo(2) casts lets Vector drain wo(1) then run the c3 reciprocal
            # chain while Scalar casts wo(2) in parallel
            emit_wo_mb(1, 6)
            emit_wo_mb(1, 7)
            emit_wo_flush(1)
            for mb in range(3, D_MODEL // P):
                emit_wo_mb(2, mb, on_scalar=True)
            emit_wo_flush(2)
            for mb in range(D_MODEL // P):
                emit_wo_mb(3, mb, on_scalar=(mb % 2 == 0))

    nc.compile()
    return nc


def _fold(w, p=P):
    # [K, M] -> [p, K/p, M] partition-major layout for contiguous DMA
    k, m = w.shape
    return np.ascontiguousarray(w.reshape(k // p, p, m).transpose(1, 0, 2))


def _prep_inputs(x, W_dq, W_uq, W_dkv, W_uk, W_uv, W_o):
    tri = np.triu(np.ones((P, P), dtype=np.float32)).astype(BF16)  # f >= p
    in_maps = []
    for c in range(8):
        b, hg = divmod(c, 4)
        cs = slice(hg * NH * D_K, (hg + 1) * NH * D_K)
        aq = np.asarray(W_dq, np.float32) @ np.asarray(W_uq, np.float32)[:, cs]
        wuk = np.asarray(W_uk, np.float32)[:, cs]
        in_maps.append({
            "xT": np.ascontiguousarray(np.asarray(x)[b].T).astype(BF16),
            "aq": _fold(aq.astype(BF16)),
            # [P, 2, NKD, P]: c-dim half major, for half-granular DMA
            "wdkv": np.ascontiguousarray(
                _fold(np.asarray(W_dkv).astype(BF16))
                .reshape(P, D_MODEL // P, 2, P).transpose(0, 2, 1, 3)),
            "wuk": _fold(wuk.astype(BF16)),
            "wuv": _fold(np.asarray(W_uv)[:, cs].astype(BF16)),
            "wo": np.asarray(W_o)[cs, :].astype(BF16).reshape(2, P, D_MODEL),
            "tri": tri,
        })
    return in_maps


def run(inputs, trace=False, **kw):
    global _cached
    if _cached is None:
        _cached = build_kernel()
    in_maps = _prep_inputs(**inputs)
    res = bass_utils.run_bass_kernel_spmd(
        _cached, in_maps, core_ids=list(range(8)), trace=trace, **kw)
    ys = [res.results[c]["yT"].astype(np.float32) for c in range(8)]
    out = np.stack([
        (ys[0] + ys[1] + ys[2] + ys[3]).T,
        (ys[4] + ys[5] + ys[6] + ys[7]).T,
    ]).astype(np.float32)
    return out, res


def kernel(**inputs):
    out, _ = run(inputs)
    return out



# revision 53
# speedup vs baseline: 1.0080x; 1.0031x over previous
"""MLA (multi-head latent attention) distributed Bass kernel for TRN2.

Full inputs in / full output out. Sharding: 8 cores = 2 batches x 4 head-groups
(4 heads each). Per-core kernel computes the latent down-projections (duplicated
across the 4 cores of a batch), up-projects Q/K/V for its 4 heads, does causal
attention in a transposed [key, query] layout (scores^T from one matmul, exp on
ScalarE with the 1/sqrt(dk) folded into the activation scale, softmax
denominator via a ones-column appended to the V stationary), and a row-sharded
W_o partial product with W_o as the stationary operand (output in [m, q]
layout). Host sums the 4 partials per batch and transposes.

Key perf facts this schedule is built around (measured via NTFF/HAM):
- The PE clock-gate (HAM) halves the PE clock unless the array sees
  sustained full-geometry activity: every matmul here presents a
  [128, x] stationary (QK uses per-head zero-padded K tiles), dummy
  matmuls pre-warm the gate during the input DMA window, and >1us PE
  gaps are scheduled away (a single idle window re-cools the gate).
- Each dma_start costs ~600ns of queue issue time, so xT moves in a
  few MB-scale slices and W_o chunks 0-2 are staged and stored with
  one DMA per chunk.
- ScalarE's queue is in-order: W_o psum->sbuf casts only go to ScalarE
  where its exp backlog is provably drained (the tail), else Vector.
- The softmax reciprocal runs on the 5x-faster approx custom-DVE op;
  operands are staged to partition-0 SBUF (the custom uop misbehaves
  on PSUM/offset operands).

Per head, attention runs kb-steps with a LAG=2 software pipeline:
QK+exp run LAG steps ahead of the AV sweep; projection/W_o pieces
interleave into the streams as 'extras' keyed by (head, cp, step).
"""

import math
import sys
import types
import numpy as np
import ml_dtypes

import concourse.bass as bass
import concourse.bacc as bacc
import concourse.mybir as mybir
import concourse.tile as tile
from concourse import bass_utils


def _harden_trace_path():
    """The agent image's antenv lacks axon_hooks and has no artifact
    bucket; if the caller enables tracing (e.g. BASS_TRACE=1), the
    bass_utils axon path would crash. Fill both gaps defensively."""
    try:
        import antenv
        try:
            import antenv.axon_hooks  # noqa: F401
        except ImportError:
            hooks = types.ModuleType("antenv.axon_hooks")
            hooks._hook = None
            hooks.set_axon_ntff_profile_hook = (
                lambda h: setattr(hooks, "_hook", h))
            hooks.get_axon_ntff_profile_hook = lambda: hooks._hook
            sys.modules["antenv.axon_hooks"] = hooks
            antenv.axon_hooks = hooks
            try:
                from trn_agent_boot.trn_boot import _ntff_profile_via_ctypes
                hook = _ntff_profile_via_ctypes("/opt/axon/libaxon_pjrt.so")
                if hook is not None:
                    hooks.set_axon_ntff_profile_hook(hook)
            except Exception:
                pass
    except ImportError:
        pass
    orig_upload = bass_utils.upload_artifacts

    def _safe_upload(tmpdir):
        try:
            return orig_upload(tmpdir)
        except Exception:
            return tmpdir

    bass_utils.upload_artifacts = _safe_upload


_harden_trace_path()

BF16 = ml_dtypes.bfloat16

D_MODEL = 1024
N_HEADS = 16
D_K = 64
D_C = 256
B, S = 2, 2048

NH = 4          # heads per core
CH = 512        # query chunk (psum bank)
NCH = S // CH   # 4 query chunks
P = 128
NKB = S // P    # 16 key blocks
INV_SQRT_DK = 1.0 / math.sqrt(D_K)

_cached = None


def build_kernel():
    nc = bacc.Bacc("TRN2", debug=False, num_devices=8)
    dt = mybir.dt
    EXP = mybir.ActivationFunctionType.Exp
    NKD = D_MODEL // P  # 8 d_model blocks

    xT_d = nc.dram_tensor("xT", [D_MODEL, S], dt.bfloat16, kind="ExternalInput")
    aq_d = nc.dram_tensor("aq", [P, NKD, NH * D_K], dt.bfloat16, kind="ExternalInput")
    wdkv_d = nc.dram_tensor("wdkv", [P, 2, NKD, P], dt.bfloat16, kind="ExternalInput")
    wuk_d = nc.dram_tensor("wuk", [P, D_C // P, NH * D_K], dt.bfloat16, kind="ExternalInput")
    wuv_d = nc.dram_tensor("wuv", [P, D_C // P, NH * D_K], dt.bfloat16, kind="ExternalInput")
    wo_d = nc.dram_tensor("wo", [2, P, D_MODEL], dt.bfloat16, kind="ExternalInput")
    tri_d = nc.dram_tensor("tri", [P, P], dt.bfloat16, kind="ExternalInput")
    # output: y^T = (x @ ... @ W_o)^T in [m, q] layout
    yT_d = nc.dram_tensor("yT", [D_MODEL, S], dt.bfloat16, kind="ExternalOutput")

    with tile.TileContext(nc) as tc:
        with (
            tc.tile_pool(name="const", bufs=1) as const,
            tc.tile_pool(name="acts", bufs=1) as acts,
            tc.tile_pool(name="exps", bufs=1) as exps,
            tc.tile_pool(name="work", bufs=4) as work,
            tc.tile_pool(name="ps", bufs=2, space="PSUM") as ps,
            tc.tile_pool(name="psa", bufs=2, space="PSUM") as psa,
            tc.tile_pool(name="psqk", bufs=2, space="PSUM") as psqk,
        ):
            xTv = xT_d.ap().rearrange("(n p) s -> p n s", p=P)
            # loads, first-needed first. Each dma_start costs ~600ns of Sync
            # queue issue time, so xT moves in 4 big column-quarter DMAs
            # (1 MB each) rather than per-d_model-block slices.
            wdkv = const.tile([P, 2, NKD, P], dt.bfloat16, tag="wdkv")
            nc.sync.dma_start(wdkv[:, 0], wdkv_d.ap()[:, 0])
            xT = const.tile([P, NKD, S], dt.bfloat16, tag="xT")
            # first chunk in 4 slices so ckv(0,0)'s k=0 matmul starts after
            # ~0.5 MB instead of a whole 1 MB quarter
            for n0 in range(0, NKD, 2):
                nc.sync.dma_start(xT[:, n0:n0 + 2, 0:CH],
                                  xTv[:, n0:n0 + 2, 0:CH])
            nc.sync.dma_start(wdkv[:, 1], wdkv_d.ap()[:, 1])
            for n0 in range(0, NKD, 4):
                nc.sync.dma_start(xT[:, n0:n0 + 4, CH:2 * CH],
                                  xTv[:, n0:n0 + 4, CH:2 * CH])
            wuk = const.tile([P, D_C // P, NH * D_K], dt.bfloat16, tag="wuk")
            nc.sync.dma_start(wuk[:], wuk_d.ap())
            aq = const.tile([P, NKD, NH * D_K], dt.bfloat16, tag="aq")
            nc.sync.dma_start(aq[:], aq_d.ap())
            wuv = const.tile([P, D_C // P, NH * D_K], dt.bfloat16, tag="wuv")
            nc.sync.dma_start(wuv[:], wuv_d.ap())
            tri = const.tile([P, P], dt.bfloat16, tag="tri")
            nc.sync.dma_start(tri[:], tri_d.ap())
            nc.sync.dma_start(xT[:, :, 2 * CH:3 * CH], xTv[:, :, 2 * CH:3 * CH])
            nc.sync.dma_start(xT[:, :, 3 * CH:S], xTv[:, :, 3 * CH:S])
            wo = []
            for n in range(2):
                t = const.tile([P, D_MODEL], dt.bfloat16, name=f"wo{n}", tag=f"wo{n}")
                nc.sync.dma_start(t[:], wo_d.ap()[n])
                wo.append(t)

            # pre-warm the PE while input DMAs are in flight: the HAM clock
            # gate needs ~3.4us of sustained full-array activity to lift the
            # PE from 1.2 to 2.4 GHz, so burn the DMA dead time on dummy
            # matmuls over an uninitialized scratch tile (result never read)
            scratch = const.tile([P, 5 * P], dt.bfloat16, tag="scratch")
            nc.gpsimd.memset(scratch[:], 1.0)
            psd = psqk.tile([P, 2 * CH], dt.float32, name="pwarm", tag="qk")
            # 14 = enough to sustain one full 3.4us HAM busy window at the
            # cold clock, without surplus warm dummies delaying real work
            for _ in range(14):
                nc.tensor.matmul(psd[:, 0:CH], scratch[:, 0:P],
                                 scratch[:, P:5 * P], start=True, stop=True)

            # persistent activations
            ckvT = [acts.tile([P, S], dt.bfloat16, name=f"ckvT{i}", tag=f"ckvT{i}")
                    for i in range(2)]
            # fp8 Q^T per head pair m: rows 64j hold head 2m+j. fp8 runs at
            # bf16 speed at this size. K^T is stored zero-padded to 128 rows
            # per head (head 2m+j in rows 64j, zeros elsewhere): the QK
            # stationary is then a full [128,128] tile (moving = both heads'
            # Q rows, zeros cancel the other head), so every attention matmul
            # drives the full PE array and the HAM clock-gate stays warm
            # (32-row DoubleRow stationaries read as idle -> 1.2 GHz).
            qTp = [acts.tile([P, S], dt.float8e4, name=f"qTp{m}", tag=f"qTp{m}")
                   for m in range(2)]
            kz = [[acts.tile([P, S], dt.float8e4, name=f"kz{m}{j}",
                             tag=f"kz{m}{j}") for j in range(2)]
                  for m in range(2)]
            for m in range(2):
                nc.gpsimd.memset(kz[m][0][D_K:P, :], 0.0)
                nc.gpsimd.memset(kz[m][1][0:D_K, :], 0.0)
            v_sb = [None] * NKB
            outT = [acts.tile([P, S], dt.bfloat16, name=f"outT{m}", tag=f"outT{m}")
                    for m in range(2)]

            # ---- single-psum projection pieces (interleavable) ----
            def emit_ckv(ch, half):
                sl = slice(ch * CH, (ch + 1) * CH)
                pp = ps.tile([P, CH], dt.float32, name="pp", tag="ps")
                for k in range(NKD):
                    nc.tensor.matmul(
                        pp[:], wdkv[:, half, k, :],
                        xT[:, k, sl], start=(k == 0), stop=(k == NKD - 1))
                nc.vector.tensor_copy(ckvT[half][:, sl], pp[:])

            def emit_k(ch, m):
                sl = slice(ch * CH, (ch + 1) * CH)
                pp = ps.tile([P, CH], dt.float32, name="pp", tag="ps")
                for half in range(2):
                    nc.tensor.matmul(
                        pp[:], wuk[:, half, m * P:(m + 1) * P],
                        ckvT[half][:, sl], start=(half == 0), stop=(half == 1))
                nc.vector.tensor_copy(kz[m][0][0:D_K, sl], pp[0:D_K, :])
                nc.vector.tensor_copy(kz[m][1][D_K:P, sl], pp[D_K:P, :])

            def emit_q(ch, m):
                sl = slice(ch * CH, (ch + 1) * CH)
                pp = ps.tile([P, CH], dt.float32, name="pp", tag="ps")
                for k in range(NKD):
                    nc.tensor.matmul(
                        pp[:], aq[:, k, m * P:(m + 1) * P],
                        xT[:, k, sl], start=(k == 0), stop=(k == NKD - 1))
                nc.vector.tensor_copy(qTp[m][:, sl], pp[:])

            def emit_v(kb):
                # V in [key, dim]: per head 64 dims + 64-wide ones block
                # (the ones columns replicate the softmax denominator to
                # psum rows 64:128 for free)
                vt = acts.tile([P, NH, 2 * D_K], dt.bfloat16,
                               name=f"v{kb}", tag=f"v{kb}")
                psv = ps.tile([P, NH * D_K], dt.float32, tag="ps")
                for half in range(2):
                    nc.tensor.matmul(
                        psv[:], ckvT[half][:, kb * P:(kb + 1) * P],
                        wuv[:, half, :], start=(half == 0), stop=(half == 1))
                nc.vector.tensor_copy(
                    vt[:, :, 0:D_K],
                    psv[:].rearrange("p (h d) -> p h d", h=NH))
                nc.gpsimd.memset(vt[:, :, D_K:2 * D_K], 1.0)
                v_sb[kb] = vt

            COPY = mybir.ActivationFunctionType.Copy
            yTv = yT_d.ap().rearrange("(n p) s -> p n s", p=P)
            # chunk-batched output staging: one DMA per query chunk for
            # chunks 0-2 (vs 8 small ones each); chunk 3 stays per-block so
            # the tail doesn't wait on a 1 MB store
            ysball = [acts.tile([P, D_MODEL // P, CH], dt.bfloat16,
                                name=f"ysb{i}", tag=f"ysb{i}") for i in range(2)]

            def emit_wo_mb(ch, mb, on_scalar=False):
                # yT[m, q] = sum_d wo[d, m] outT[d, q]: one (m, q-chunk) block
                sl = slice(ch * CH, (ch + 1) * CH)
                if ch == 3:
                    ysb = work.tile([P, CH], dt.bfloat16,
                                    name="ysb", tag="ysb")[:]
                else:
                    ysb = ysball[ch % 2][:, mb, :]
                pp = ps.tile([P, CH], dt.float32, name="pp", tag="ps")
                for db in range(2):
                    nc.tensor.matmul(
                        pp[:], wo[db][:, mb * P:(mb + 1) * P],
                        outT[db][:, sl], start=(db == 0), stop=(db == 1))
                if on_scalar:  # ScalarE takes casts when exp leaves it idle
                    nc.scalar.activation(ysb, pp[:], COPY)
                else:
                    nc.vector.tensor_copy(ysb, pp[:])
                if ch == 3:
                    nc.sync.dma_start(yT_d.ap()[mb * P:(mb + 1) * P, sl], ysb)

            def emit_wo_flush(ch):
                sl = slice(ch * CH, (ch + 1) * CH)
                nc.sync.dma_start(yTv[:, :, sl], ysball[ch % 2][:])

            # minimal pre-attention pass: just what head 0 cp0 needs
            # (pair-0 q/k for queries/keys 0:1024); the rest interleaves
            # into the attention streams below. Piece order tracks DMA
            # arrival so the PE never gaps (a >.5us gap can re-cool the
            # HAM clock gate).
            # q pieces depend only on xT/aq (not the ckvT casts), so they
            # slot between ckv and k pieces to cover Vector cast latency
            emit_ckv(0, 0)
            emit_ckv(0, 1)
            emit_ckv(1, 0)
            emit_ckv(1, 1)
            emit_q(0, 0)
            emit_k(0, 0)
            emit_q(1, 0)
            emit_k(1, 0)

            # per-(head, cp) extras: {h: {cp: {kb: [fns]}}}
            extras_map = {h: {0: {}, 1: {}} for h in range(NH)}

            def put(h, cp, kb, fn):
                extras_map[h][cp].setdefault(kb, []).append(fn)

            # h0 cp0: V blocks 0-7 (AV needs v[kb] at step kb+LAG) and
            # pair-0 queries 1024:2048 (h0 cp1 moving operand)
            for kb in range(8):
                put(0, 0, kb, lambda kb=kb: emit_v(kb))
            put(0, 0, 4, lambda: emit_q(2, 0))
            put(0, 0, 6, lambda: emit_q(3, 0))
            # h0 cp1: latent chunks 2-3 + pair-0 keys 1024:2048 (needed at
            # kb8/kb12) + V blocks 8-15
            pieces = [lambda: emit_ckv(2, 0), lambda: emit_ckv(2, 1),
                      lambda: emit_k(2, 0), lambda: emit_ckv(3, 0),
                      lambda: emit_ckv(3, 1), lambda: emit_k(3, 0)]
            for i, pc in enumerate(pieces):
                put(0, 1, i, pc)
            for kb in range(8, NKB):
                put(0, 1, kb, lambda kb=kb: emit_v(kb))
            # h1: pair-1 q/k (heads 2,3 start at h2)
            put(1, 0, 0, lambda: emit_k(0, 1))
            put(1, 0, 1, lambda: emit_q(0, 1))
            put(1, 0, 2, lambda: emit_k(1, 1))
            put(1, 0, 3, lambda: emit_q(1, 1))
            put(1, 1, 0, lambda: emit_k(2, 1))
            put(1, 1, 1, lambda: emit_q(2, 1))
            put(1, 1, 2, lambda: emit_k(3, 1))
            put(1, 1, 3, lambda: emit_q(3, 1))

            # ---- attention: per head, chunk-pair major, QK/exp ahead of a
            # lagged AV sweep; denominator rows 64:128 of psav ----
            LAG = 2
            for h in range(NH):
                ht, off = divmod(h, 2)
                q_h = qTp[h // 2]
                k_h = kz[h // 2][h % 2]
                es_tiles = [None] * NKB
                psav = [None] * NCH

                def emit_qk(kb, cp):
                    q0 = P * kb       # first valid query for this key block
                    pq0 = 1024 * cp   # pair covers q in [pq0, pq0+1024)
                    if es_tiles[kb] is None:
                        es_tiles[kb] = exps.tile(
                            [P, S - q0], dt.bfloat16,
                            name=f"es{kb}", tag=f"es{kb}")
                    es = es_tiles[kb]
                    lo = max(q0, pq0)
                    pqk = psqk.tile([P, 2 * CH], dt.float32,
                                    name="pqk", tag="qk")
                    for ch in (2 * cp, 2 * cp + 1):
                        clo = max(q0, ch * CH)
                        if clo >= (ch + 1) * CH:
                            continue
                        nc.tensor.matmul(
                            pqk[:, clo - pq0:(ch + 1) * CH - pq0],
                            k_h[:, q0:q0 + P],
                            q_h[:, clo:(ch + 1) * CH],
                            start=True, stop=True)
                    nc.scalar.activation(
                        es[:, lo - q0:pq0 + 2 * CH - q0],
                        pqk[:, lo - pq0:2 * CH],
                        EXP, scale=INV_SQRT_DK)
                    if cp == kb // 8:
                        # mask the diagonal [128, 128] triangle (valid f >= p)
                        nc.vector.tensor_mul(es[:, 0:P], es[:, 0:P], tri[:])

                def emit_av(kb, cp):
                    q0 = P * kb
                    for c in (2 * cp, 2 * cp + 1):
                        if kb // 4 > c:
                            continue
                        n0 = max(q0 - CH * c, 0)
                        nc.tensor.matmul(
                            psav[c][:, n0:CH], v_sb[kb][:, h, :],
                            es_tiles[kb][:, CH * c + n0 - q0:
                                         CH * (c + 1) - q0],
                            start=(kb == 0), stop=(kb == 4 * c + 3))
                        if kb == 4 * c + 3:  # chunk done -> normalize
                            # approx reciprocal (~5x faster than exact); the
                            # custom-DVE op wants SBUF operands at partition
                            # 0, so stage the denominators through rb first
                            # (on Scalar for the last head -- its normalize
                            # chains gate the W_o tail and Vector is busy)
                            rb = work.tile([D_K, CH], dt.float32, tag="rb")
                            if h == NH - 1 and c == 3:
                                # Scalar's exp queue is drained by now, and
                                # Vector is busy with W_o casts: staging on
                                # Scalar unblocks the final W_o chunk sooner
                                nc.scalar.activation(
                                    rb[:], psav[c][D_K:2 * D_K, :], COPY)
                            else:
                                nc.vector.tensor_copy(
                                    rb[:], psav[c][D_K:2 * D_K, :])
                            nc.vector.reciprocal_approx_fast(rb[:], rb[:])
                            nc.vector.tensor_mul(
                                outT[ht][off * D_K:(off + 1) * D_K,
                                         c * CH:(c + 1) * CH],
                                psav[c][0:D_K, :], rb[:])

                for cp in range(2):
                    for c in (2 * cp, 2 * cp + 1):
                        psav[c] = psa.tile([P, CH], dt.float32,
                                           name="psav", tag="psa")
                    kmax = 8 * cp + 8
                    extras = extras_map[h][cp]
                    if h == NH - 1 and cp == 1:
                        # W_o rides along head 3 cp1: chunks 0,1 are fully
                        # normalized after h3 cp0; chunk 2 after step 13.
                        # A few ready pieces are held back past the loop so
                        # the PE stays busy while psav[3] normalizes (a >1us
                        # gap would re-cool the HAM clock gate for the tail).
                        jobs = [(c, mb) for c in (0, 1)
                                for mb in range(D_MODEL // P)][:14]
                        for kb, job in zip(range(1, 15), jobs):
                            # all on Vector: a Scalar cast here would queue
                            # in front of this head's remaining exps
                            extras.setdefault(kb, []).append(
                                lambda job=job: emit_wo_mb(job[0], job[1]))
                        extras.setdefault(9, []).append(
                            lambda: emit_wo_flush(0))
                        for i, mb in enumerate(range(3)):
                            extras.setdefault(15 + i, []).append(
                                lambda mb=mb: emit_wo_mb(2, mb))
                    for kb in range(kmax + LAG):
                        for fn in extras.get(kb, ()):
                            fn()
                        if kb < kmax:
                            emit_qk(kb, cp)
                        if kb >= LAG:
                            emit_av(kb - LAG, cp)
            # dep-free pieces first: they overlap psav[3]'s normalize chain.
            # Alternate the psum->sbuf casts between Scalar (idle here) and
            # Vector so the cast is never the tail's rate limiter.
            # Scalar's queue here is [rb-c3 staging copy, ...]: giving it the
            # wo(2) casts lets Vector drain wo(1) then run the c3 reciprocal
            # chain while Scalar casts wo(2) in parallel
            emit_wo_mb(1, 6)
            emit_wo_mb(1, 7)
            emit_wo_flush(1)
            for mb in range(3, D_MODEL // P):
                emit_wo_mb(2, mb, on_scalar=True)
            emit_wo_flush(2)
            for mb in range(D_MODEL // P):
                emit_wo_mb(3, mb, on_scalar=(mb % 2 == 0))

    nc.compile()
    return nc


def _fold(w, p=P):
    # [K, M] -> [p, K/p, M] partition-major layout for contiguous DMA
    k, m = w.shape
    return np.ascontiguousarray(w.reshape(k // p, p, m).transpose(1, 0, 2))


def _prep_inputs(x, W_dq, W_uq, W_dkv, W_uk, W_uv, W_o):
    tri = np.triu(np.ones((P, P), dtype=np.float32)).astype(BF16)  # f >= p
    in_maps = []
    for c in range(8):
        b, hg = divmod(c, 4)
        cs = slice(hg * NH * D_K, (hg + 1) * NH * D_K)
        aq = np.asarray(W_dq, np.float32) @ np.asarray(W_uq, np.float32)[:, cs]
        wuk = np.asarray(W_uk, np.float32)[:, cs]
        in_maps.append({
            "xT": np.ascontiguousarray(np.asarray(x)[b].T).astype(BF16),
            "aq": _fold(aq.astype(BF16)),
            # [P, 2, NKD, P]: c-dim half major, for half-granular DMA
            "wdkv": np.ascontiguousarray(
                _fold(np.asarray(W_dkv).astype(BF16))
                .reshape(P, D_MODEL // P, 2, P).transpose(0, 2, 1, 3)),
            "wuk": _fold(wuk.astype(BF16)),
            "wuv": _fold(np.asarray(W_uv)[:, cs].astype(BF16)),
            "wo": np.asarray(W_o)[cs, :].astype(BF16).reshape(2, P, D_MODEL),
            "tri": tri,
        })
    return in_maps


def run(inputs, trace=False, **kw):
    global _cached
    if _cached is None:
        _cached = build_kernel()
    in_maps = _prep_inputs(**inputs)
    res = bass_utils.run_bass_kernel_spmd(
        _cached, in_maps, core_ids=list(range(8)), trace=trace, **kw)
    ys = [res.results[c]["yT"].astype(np.float32) for c in range(8)]
    out = np.stack([
        (ys[0] + ys[1] + ys[2] + ys[3]).T,
        (ys[4] + ys[5] + ys[6] + ys[7]).T,
    ]).astype(np.float32)
    return out, res


def kernel(**inputs):
    out, _ = run(inputs)
    return out



# revision 54
# speedup vs baseline: 1.0157x; 1.0076x over previous
"""MLA (multi-head latent attention) distributed Bass kernel for TRN2.

Full inputs in / full output out. Sharding: 8 cores = 2 batches x 4 head-groups
(4 heads each). Per-core kernel computes the latent down-projections (duplicated
across the 4 cores of a batch), up-projects Q/K/V for its 4 heads, does causal
attention in a transposed [key, query] layout (scores^T from one matmul, exp on
ScalarE with the 1/sqrt(dk) folded into the activation scale, softmax
denominator via a ones-column appended to the V stationary), and a row-sharded
W_o partial product with W_o as the stationary operand (output in [m, q]
layout). Host sums the 4 partials per batch and transposes.

Key perf facts this schedule is built around (measured via NTFF/HAM):
- The PE clock-gate (HAM) halves the PE clock unless the array sees
  sustained full-geometry activity: every matmul here presents a
  [128, x] stationary (QK uses per-head zero-padded K tiles), dummy
  matmuls pre-warm the gate during the input DMA window, and >1us PE
  gaps are scheduled away (a single idle window re-cools the gate).
- Each dma_start costs ~600ns of queue issue time, so xT moves in a
  few MB-scale slices and W_o chunks 0-2 are staged and stored with
  one DMA per chunk.
- ScalarE's queue is in-order: W_o psum->sbuf casts only go to ScalarE
  where its exp backlog is provably drained (the tail), else Vector.
- The softmax reciprocal runs on the 5x-faster approx custom-DVE op;
  operands are staged to partition-0 SBUF (the custom uop misbehaves
  on PSUM/offset operands).

Per head, attention runs kb-steps with a LAG=2 software pipeline:
QK+exp run LAG steps ahead of the AV sweep; projection/W_o pieces
interleave into the streams as 'extras' keyed by (head, cp, step).
"""

import math
import sys
import types
import numpy as np
import ml_dtypes

import concourse.bass as bass
import concourse.bacc as bacc
import concourse.mybir as mybir
import concourse.tile as tile
from concourse import bass_utils


def _harden_trace_path():
    """The agent image's antenv lacks axon_hooks and has no artifact
    bucket; if the caller enables tracing (e.g. BASS_TRACE=1), the
    bass_utils axon path would crash. Fill both gaps defensively."""
    try:
        import antenv
        try:
            import antenv.axon_hooks  # noqa: F401
        except ImportError:
            hooks = types.ModuleType("antenv.axon_hooks")
            hooks._hook = None
            hooks.set_axon_ntff_profile_hook = (
                lambda h: setattr(hooks, "_hook", h))
            hooks.get_axon_ntff_profile_hook = lambda: hooks._hook
            sys.modules["antenv.axon_hooks"] = hooks
            antenv.axon_hooks = hooks
            try:
                from trn_agent_boot.trn_boot import _ntff_profile_via_ctypes
                hook = _ntff_profile_via_ctypes("/opt/axon/libaxon_pjrt.so")
                if hook is not None:
                    hooks.set_axon_ntff_profile_hook(hook)
            except Exception:
                pass
    except ImportError:
        pass
    orig_upload = bass_utils.upload_artifacts

    def _safe_upload(tmpdir):
        try:
            return orig_upload(tmpdir)
        except Exception:
            return tmpdir

    bass_utils.upload_artifacts = _safe_upload


_harden_trace_path()

BF16 = ml_dtypes.bfloat16

D_MODEL = 1024
N_HEADS = 16
D_K = 64
D_C = 256
B, S = 2, 2048

NH = 4          # heads per core
CH = 512        # query chunk (psum bank)
NCH = S // CH   # 4 query chunks
P = 128
NKB = S // P    # 16 key blocks
INV_SQRT_DK = 1.0 / math.sqrt(D_K)

_cached = None


def build_kernel():
    nc = bacc.Bacc("TRN2", debug=False, num_devices=8)
    dt = mybir.dt
    EXP = mybir.ActivationFunctionType.Exp
    NKD = D_MODEL // P  # 8 d_model blocks

    xT_d = nc.dram_tensor("xT", [D_MODEL, S], dt.bfloat16, kind="ExternalInput")
    aq_d = nc.dram_tensor("aq", [P, NKD, NH * D_K], dt.bfloat16, kind="ExternalInput")
    wdkv_d = nc.dram_tensor("wdkv", [P, 2, NKD, P], dt.bfloat16, kind="ExternalInput")
    wuk_d = nc.dram_tensor("wuk", [P, D_C // P, NH * D_K], dt.bfloat16, kind="ExternalInput")
    wuv_d = nc.dram_tensor("wuv", [P, D_C // P, NH * D_K], dt.bfloat16, kind="ExternalInput")
    wo_d = nc.dram_tensor("wo", [2, P, D_MODEL], dt.bfloat16, kind="ExternalInput")
    tri_d = nc.dram_tensor("tri", [P, P], dt.bfloat16, kind="ExternalInput")
    # output: y^T = (x @ ... @ W_o)^T in [m, q] layout
    yT_d = nc.dram_tensor("yT", [D_MODEL, S], dt.bfloat16, kind="ExternalOutput")

    with tile.TileContext(nc) as tc:
        with (
            tc.tile_pool(name="const", bufs=1) as const,
            tc.tile_pool(name="acts", bufs=1) as acts,
            tc.tile_pool(name="exps", bufs=1) as exps,
            tc.tile_pool(name="work", bufs=4) as work,
            tc.tile_pool(name="ps", bufs=2, space="PSUM") as ps,
            tc.tile_pool(name="psa", bufs=2, space="PSUM") as psa,
            tc.tile_pool(name="psqk", bufs=2, space="PSUM") as psqk,
        ):
            xTv = xT_d.ap().rearrange("(n p) s -> p n s", p=P)
            # loads, first-needed first. Each dma_start costs ~600ns of Sync
            # queue issue time, so xT moves in 4 big column-quarter DMAs
            # (1 MB each) rather than per-d_model-block slices.
            wdkv = const.tile([P, 2, NKD, P], dt.bfloat16, tag="wdkv")
            nc.sync.dma_start(wdkv[:, 0], wdkv_d.ap()[:, 0])
            xT = const.tile([P, NKD, S], dt.bfloat16, tag="xT")
            # first chunk in 4 slices so ckv(0,0)'s k=0 matmul starts after
            # ~0.5 MB instead of a whole 1 MB quarter
            for n0 in range(0, NKD, 2):
                nc.sync.dma_start(xT[:, n0:n0 + 2, 0:CH],
                                  xTv[:, n0:n0 + 2, 0:CH])
            nc.sync.dma_start(wdkv[:, 1], wdkv_d.ap()[:, 1])
            for n0 in range(0, NKD, 4):
                nc.sync.dma_start(xT[:, n0:n0 + 4, CH:2 * CH],
                                  xTv[:, n0:n0 + 4, CH:2 * CH])
            aq = const.tile([P, NKD, NH * D_K], dt.bfloat16, tag="aq")
            nc.sync.dma_start(aq[:], aq_d.ap())
            wuk = const.tile([P, D_C // P, NH * D_K], dt.bfloat16, tag="wuk")
            nc.sync.dma_start(wuk[:], wuk_d.ap())
            wuv = const.tile([P, D_C // P, NH * D_K], dt.bfloat16, tag="wuv")
            nc.sync.dma_start(wuv[:], wuv_d.ap())
            tri = const.tile([P, P], dt.bfloat16, tag="tri")
            nc.sync.dma_start(tri[:], tri_d.ap())
            nc.sync.dma_start(xT[:, :, 2 * CH:3 * CH], xTv[:, :, 2 * CH:3 * CH])
            nc.sync.dma_start(xT[:, :, 3 * CH:S], xTv[:, :, 3 * CH:S])
            wo = []
            for n in range(2):
                t = const.tile([P, D_MODEL], dt.bfloat16, name=f"wo{n}", tag=f"wo{n}")
                nc.sync.dma_start(t[:], wo_d.ap()[n])
                wo.append(t)

            # pre-warm the PE while input DMAs are in flight: the HAM clock
            # gate needs ~3.4us of sustained full-array activity to lift the
            # PE from 1.2 to 2.4 GHz, so burn the DMA dead time on dummy
            # matmuls over an uninitialized scratch tile (result never read)
            scratch = const.tile([P, 5 * P], dt.bfloat16, tag="scratch")
            nc.gpsimd.memset(scratch[:], 1.0)
            psd = psqk.tile([P, 2 * CH], dt.float32, name="pwarm", tag="qk")
            # 14 = enough to sustain one full 3.4us HAM busy window at the
            # cold clock, without surplus warm dummies delaying real work
            for _ in range(14):
                nc.tensor.matmul(psd[:, 0:CH], scratch[:, 0:P],
                                 scratch[:, P:5 * P], start=True, stop=True)

            # persistent activations
            ckvT = [acts.tile([P, S], dt.bfloat16, name=f"ckvT{i}", tag=f"ckvT{i}")
                    for i in range(2)]
            # fp8 Q^T per head pair m: rows 64j hold head 2m+j. fp8 runs at
            # bf16 speed at this size. K^T is stored zero-padded to 128 rows
            # per head (head 2m+j in rows 64j, zeros elsewhere): the QK
            # stationary is then a full [128,128] tile (moving = both heads'
            # Q rows, zeros cancel the other head), so every attention matmul
            # drives the full PE array and the HAM clock-gate stays warm
            # (32-row DoubleRow stationaries read as idle -> 1.2 GHz).
            qTp = [acts.tile([P, S], dt.float8e4, name=f"qTp{m}", tag=f"qTp{m}")
                   for m in range(2)]
            kz = [[acts.tile([P, S], dt.float8e4, name=f"kz{m}{j}",
                             tag=f"kz{m}{j}") for j in range(2)]
                  for m in range(2)]
            for m in range(2):
                nc.gpsimd.memset(kz[m][0][D_K:P, :], 0.0)
                nc.gpsimd.memset(kz[m][1][0:D_K, :], 0.0)
            v_sb = [None] * NKB
            outT = [acts.tile([P, S], dt.bfloat16, name=f"outT{m}", tag=f"outT{m}")
                    for m in range(2)]

            # ---- single-psum projection pieces (interleavable) ----
            def emit_ckv(ch, half):
                sl = slice(ch * CH, (ch + 1) * CH)
                pp = ps.tile([P, CH], dt.float32, name="pp", tag="ps")
                for k in range(NKD):
                    nc.tensor.matmul(
                        pp[:], wdkv[:, half, k, :],
                        xT[:, k, sl], start=(k == 0), stop=(k == NKD - 1))
                nc.vector.tensor_copy(ckvT[half][:, sl], pp[:])

            def emit_k(ch, m):
                sl = slice(ch * CH, (ch + 1) * CH)
                pp = ps.tile([P, CH], dt.float32, name="pp", tag="ps")
                for half in range(2):
                    nc.tensor.matmul(
                        pp[:], wuk[:, half, m * P:(m + 1) * P],
                        ckvT[half][:, sl], start=(half == 0), stop=(half == 1))
                nc.vector.tensor_copy(kz[m][0][0:D_K, sl], pp[0:D_K, :])
                nc.vector.tensor_copy(kz[m][1][D_K:P, sl], pp[D_K:P, :])

            def emit_q(ch, m):
                sl = slice(ch * CH, (ch + 1) * CH)
                pp = ps.tile([P, CH], dt.float32, name="pp", tag="ps")
                for k in range(NKD):
                    nc.tensor.matmul(
                        pp[:], aq[:, k, m * P:(m + 1) * P],
                        xT[:, k, sl], start=(k == 0), stop=(k == NKD - 1))
                nc.vector.tensor_copy(qTp[m][:, sl], pp[:])

            def emit_v(kb):
                # V in [key, dim]: per head 64 dims + 64-wide ones block
                # (the ones columns replicate the softmax denominator to
                # psum rows 64:128 for free)
                vt = acts.tile([P, NH, 2 * D_K], dt.bfloat16,
                               name=f"v{kb}", tag=f"v{kb}")
                psv = ps.tile([P, NH * D_K], dt.float32, tag="ps")
                for half in range(2):
                    nc.tensor.matmul(
                        psv[:], ckvT[half][:, kb * P:(kb + 1) * P],
                        wuv[:, half, :], start=(half == 0), stop=(half == 1))
                nc.vector.tensor_copy(
                    vt[:, :, 0:D_K],
                    psv[:].rearrange("p (h d) -> p h d", h=NH))
                nc.gpsimd.memset(vt[:, :, D_K:2 * D_K], 1.0)
                v_sb[kb] = vt

            COPY = mybir.ActivationFunctionType.Copy
            yTv = yT_d.ap().rearrange("(n p) s -> p n s", p=P)
            # chunk-batched output staging: one DMA per query chunk for
            # chunks 0-2 (vs 8 small ones each); chunk 3 stays per-block so
            # the tail doesn't wait on a 1 MB store
            ysball = [acts.tile([P, D_MODEL // P, CH], dt.bfloat16,
                                name=f"ysb{i}", tag=f"ysb{i}") for i in range(2)]

            def emit_wo_mb(ch, mb, on_scalar=False):
                # yT[m, q] = sum_d wo[d, m] outT[d, q]: one (m, q-chunk) block
                sl = slice(ch * CH, (ch + 1) * CH)
                if ch == 3:
                    ysb = work.tile([P, CH], dt.bfloat16,
                                    name="ysb", tag="ysb")[:]
                else:
                    ysb = ysball[ch % 2][:, mb, :]
                pp = ps.tile([P, CH], dt.float32, name="pp", tag="ps")
                for db in range(2):
                    nc.tensor.matmul(
                        pp[:], wo[db][:, mb * P:(mb + 1) * P],
                        outT[db][:, sl], start=(db == 0), stop=(db == 1))
                if on_scalar:  # ScalarE takes casts when exp leaves it idle
                    nc.scalar.activation(ysb, pp[:], COPY)
                else:
                    nc.vector.tensor_copy(ysb, pp[:])
                if ch == 3:
                    nc.sync.dma_start(yT_d.ap()[mb * P:(mb + 1) * P, sl], ysb)

            def emit_wo_flush(ch):
                sl = slice(ch * CH, (ch + 1) * CH)
                nc.sync.dma_start(yTv[:, :, sl], ysball[ch % 2][:])

            # minimal pre-attention pass: just what head 0 cp0 needs
            # (pair-0 q/k for queries/keys 0:1024); the rest interleaves
            # into the attention streams below. Piece order tracks DMA
            # arrival so the PE never gaps (a >.5us gap can re-cool the
            # HAM clock gate).
            # q pieces depend only on xT/aq (not the ckvT casts), so they
            # slot between ckv and k pieces to cover Vector cast latency
            emit_ckv(0, 0)
            emit_ckv(0, 1)
            emit_ckv(1, 0)
            emit_ckv(1, 1)
            emit_q(0, 0)
            emit_k(0, 0)
            emit_q(1, 0)
            emit_k(1, 0)

            # per-(head, cp) extras: {h: {cp: {kb: [fns]}}}
            extras_map = {h: {0: {}, 1: {}} for h in range(NH)}

            def put(h, cp, kb, fn):
                extras_map[h][cp].setdefault(kb, []).append(fn)

            # h0 cp0: V blocks 0-7 (AV needs v[kb] at step kb+LAG) and
            # pair-0 queries 1024:2048 (h0 cp1 moving operand)
            for kb in range(8):
                put(0, 0, kb, lambda kb=kb: emit_v(kb))
            put(0, 0, 4, lambda: emit_q(2, 0))
            put(0, 0, 6, lambda: emit_q(3, 0))
            # h0 cp1: latent chunks 2-3 + pair-0 keys 1024:2048 (needed at
            # kb8/kb12) + V blocks 8-15
            pieces = [lambda: emit_ckv(2, 0), lambda: emit_ckv(2, 1),
                      lambda: emit_k(2, 0), lambda: emit_ckv(3, 0),
                      lambda: emit_ckv(3, 1), lambda: emit_k(3, 0)]
            for i, pc in enumerate(pieces):
                put(0, 1, i, pc)
            for kb in range(8, NKB):
                put(0, 1, kb, lambda kb=kb: emit_v(kb))
            # h1: pair-1 q/k (heads 2,3 start at h2)
            put(1, 0, 0, lambda: emit_k(0, 1))
            put(1, 0, 1, lambda: emit_q(0, 1))
            put(1, 0, 2, lambda: emit_k(1, 1))
            put(1, 0, 3, lambda: emit_q(1, 1))
            put(1, 1, 0, lambda: emit_k(2, 1))
            put(1, 1, 1, lambda: emit_q(2, 1))
            put(1, 1, 2, lambda: emit_k(3, 1))
            put(1, 1, 3, lambda: emit_q(3, 1))

            # ---- attention: per head, chunk-pair major, QK/exp ahead of a
            # lagged AV sweep; denominator rows 64:128 of psav ----
            LAG = 2
            for h in range(NH):
                ht, off = divmod(h, 2)
                q_h = qTp[h // 2]
                k_h = kz[h // 2][h % 2]
                es_tiles = [None] * NKB
                psav = [None] * NCH

                def emit_qk(kb, cp):
                    q0 = P * kb       # first valid query for this key block
                    pq0 = 1024 * cp   # pair covers q in [pq0, pq0+1024)
                    if es_tiles[kb] is None:
                        es_tiles[kb] = exps.tile(
                            [P, S - q0], dt.bfloat16,
                            name=f"es{kb}", tag=f"es{kb}")
                    es = es_tiles[kb]
                    lo = max(q0, pq0)
                    pqk = psqk.tile([P, 2 * CH], dt.float32,
                                    name="pqk", tag="qk")
                    for ch in (2 * cp, 2 * cp + 1):
                        clo = max(q0, ch * CH)
                        if clo >= (ch + 1) * CH:
                            continue
                        nc.tensor.matmul(
                            pqk[:, clo - pq0:(ch + 1) * CH - pq0],
                            k_h[:, q0:q0 + P],
                            q_h[:, clo:(ch + 1) * CH],
                            start=True, stop=True)
                    nc.scalar.activation(
                        es[:, lo - q0:pq0 + 2 * CH - q0],
                        pqk[:, lo - pq0:2 * CH],
                        EXP, scale=INV_SQRT_DK)
                    if cp == kb // 8:
                        # mask the diagonal [128, 128] triangle (valid f >= p)
                        nc.vector.tensor_mul(es[:, 0:P], es[:, 0:P], tri[:])

                def emit_av(kb, cp):
                    q0 = P * kb
                    for c in (2 * cp, 2 * cp + 1):
                        if kb // 4 > c:
                            continue
                        n0 = max(q0 - CH * c, 0)
                        nc.tensor.matmul(
                            psav[c][:, n0:CH], v_sb[kb][:, h, :],
                            es_tiles[kb][:, CH * c + n0 - q0:
                                         CH * (c + 1) - q0],
                            start=(kb == 0), stop=(kb == 4 * c + 3))
                        if kb == 4 * c + 3:  # chunk done -> normalize
                            # approx reciprocal (~5x faster than exact); the
                            # custom-DVE op wants SBUF operands at partition
                            # 0, so stage the denominators through rb first
                            # (on Scalar for the last head -- its normalize
                            # chains gate the W_o tail and Vector is busy)
                            rb = work.tile([D_K, CH], dt.float32, tag="rb")
                            if h == NH - 1 and c == 3:
                                # Scalar's exp queue is drained by now, and
                                # Vector is busy with W_o casts: staging on
                                # Scalar unblocks the final W_o chunk sooner
                                nc.scalar.activation(
                                    rb[:], psav[c][D_K:2 * D_K, :], COPY)
                            else:
                                nc.vector.tensor_copy(
                                    rb[:], psav[c][D_K:2 * D_K, :])
                            nc.vector.reciprocal_approx_fast(rb[:], rb[:])
                            nc.vector.tensor_mul(
                                outT[ht][off * D_K:(off + 1) * D_K,
                                         c * CH:(c + 1) * CH],
                                psav[c][0:D_K, :], rb[:])

                for cp in range(2):
                    for c in (2 * cp, 2 * cp + 1):
                        psav[c] = psa.tile([P, CH], dt.float32,
                                           name="psav", tag="psa")
                    kmax = 8 * cp + 8
                    extras = extras_map[h][cp]
                    if h == NH - 1 and cp == 1:
                        # W_o rides along head 3 cp1: chunks 0,1 are fully
                        # normalized after h3 cp0; chunk 2 after step 13.
                        # A few ready pieces are held back past the loop so
                        # the PE stays busy while psav[3] normalizes (a >1us
                        # gap would re-cool the HAM clock gate for the tail).
                        jobs = [(c, mb) for c in (0, 1)
                                for mb in range(D_MODEL // P)][:14]
                        for kb, job in zip(range(1, 15), jobs):
                            # all on Vector: a Scalar cast here would queue
                            # in front of this head's remaining exps
                            extras.setdefault(kb, []).append(
                                lambda job=job: emit_wo_mb(job[0], job[1]))
                        extras.setdefault(9, []).append(
                            lambda: emit_wo_flush(0))
                        for i, mb in enumerate(range(3)):
                            extras.setdefault(15 + i, []).append(
                                lambda mb=mb: emit_wo_mb(2, mb))
                    for kb in range(kmax + LAG):
                        for fn in extras.get(kb, ()):
                            fn()
                        if kb < kmax:
                            emit_qk(kb, cp)
                        if kb >= LAG:
                            emit_av(kb - LAG, cp)
            # dep-free pieces first: they overlap psav[3]'s normalize chain.
            # Alternate the psum->sbuf casts between Scalar (idle here) and
            # Vector so the cast is never the tail's rate limiter.
            # Scalar's queue here is [rb-c3 staging copy, ...]: giving it the
            # wo(2) casts lets Vector drain wo(1) then run the c3 reciprocal
            # chain while Scalar casts wo(2) in parallel
            emit_wo_mb(1, 6)
            emit_wo_mb(1, 7)
            emit_wo_flush(1)
            for mb in range(3, D_MODEL // P):
                emit_wo_mb(2, mb, on_scalar=True)
            emit_wo_flush(2)
            for mb in range(D_MODEL // P):
                emit_wo_mb(3, mb, on_scalar=(mb % 2 == 0))

    nc.compile()
    return nc


def _fold(w, p=P):
    # [K, M] -> [p, K/p, M] partition-major layout for contiguous DMA
    k, m = w.shape
    return np.ascontiguousarray(w.reshape(k // p, p, m).transpose(1, 0, 2))


def _prep_inputs(x, W_dq, W_uq, W_dkv, W_uk, W_uv, W_o):
    tri = np.triu(np.ones((P, P), dtype=np.float32)).astype(BF16)  # f >= p
    in_maps = []
    for c in range(8):
        b, hg = divmod(c, 4)
        cs = slice(hg * NH * D_K, (hg + 1) * NH * D_K)
        aq = np.asarray(W_dq, np.float32) @ np.asarray(W_uq, np.float32)[:, cs]
        wuk = np.asarray(W_uk, np.float32)[:, cs]
        in_maps.append({
            "xT": np.ascontiguousarray(np.asarray(x)[b].T).astype(BF16),
            "aq": _fold(aq.astype(BF16)),
            # [P, 2, NKD, P]: c-dim half major, for half-granular DMA
            "wdkv": np.ascontiguousarray(
                _fold(np.asarray(W_dkv).astype(BF16))
                .reshape(P, D_MODEL // P, 2, P).transpose(0, 2, 1, 3)),
            "wuk": _fold(wuk.astype(BF16)),
            "wuv": _fold(np.asarray(W_uv)[:, cs].astype(BF16)),
            "wo": np.asarray(W_o)[cs, :].astype(BF16).reshape(2, P, D_MODEL),
            "tri": tri,
        })
    return in_maps


def run(inputs, trace=False, **kw):
    global _cached
    if _cached is None:
        _cached = build_kernel()
    in_maps = _prep_inputs(**inputs)
    res = bass_utils.run_bass_kernel_spmd(
        _cached, in_maps, core_ids=list(range(8)), trace=trace, **kw)
    ys = [res.results[c]["yT"].astype(np.float32) for c in range(8)]
    out = np.stack([
        (ys[0] + ys[1] + ys[2] + ys[3]).T,
        (ys[4] + ys[5] + ys[6] + ys[7]).T,
    ]).astype(np.float32)
    return out, res


def kernel(**inputs):
    out, _ = run(inputs)
    return out

